# revision 37
# baseline (speedup 1.0000x reference)
"""Depthwise causal conv1d kernel for Trainium2 (8 NeuronCores, SPMD).

Problem: x [B=8, T=4096, C=512] f32, weight [C=512, K=4] f32.
out[b, t, c] = sum_k weight[c, k] * x[b, t - 3 + k, c]   (causal, zero-pad)

Strategy (v6 — engine-split, warmed PE, 2048-col PSUM quads):
  - Data-parallel over batch: core b handles x[b].
  - Host-side layout: channels-first x[b].T padded with 3 leading zeros
    along time -> [C=512, T+3=4099] fp16, chunk-blocked to [128, 4*4099].
    fp16 halves HBM traffic; accumulation is fp32 in PSUM / DVE ALU.
  - Diag stationary matrices are built on the HOST and DMA'd.
  - Work split: PE does taps 0-2 as accumulating diag matmuls (chunk 2's
    quads run only taps 0-1 on the PE, with taps 2-3 on DVE, trimming
    the PE's critical path), DVE does tap 3 via tensor_scalar_mul (4x
    fp16 mode) plus the final f16 adds (2x mode), ACT evacuates
    PSUM->f16 in 2048-col instructions. PSUM = 2 x [128, 2048] quads =
    all 8 banks.
  - A short burst of dummy matmuls (fed by the first-emitted DVE memset,
    writing the pre-allocated first PSUM quad) ramps the PE p-state to
    full speed while the input DMAs are in flight, so every real matmul
    runs at ~216 ns instead of paying the 3 us mid-speed ramp.
  - Tail: chunk3-q0 runs all 4 taps on PE with an ACT-only evacuation
    (copy + ACT-DGE trigger); the last quad evacuates via 4x 512-col
    scalar_tensor_tensor (out = x3*w3 + psum) into a DEDICATED output
    tile (avoids a tile-ordering false dependency on chunk3's other
    output) with sync-DGE triggers, which by then are idle.
"""

import numpy as np

B, T, C, K = 8, 4096, 512, 4
P = 128  # partitions
NCHUNK = C // P  # 4 channel chunks
TJ = 512  # time-tile (free dim) per matmul; max moving free dim
NJ = T // TJ  # 8 time tiles per chunk
TP = T + K - 1  # padded time = 4099
KPE = 3  # taps 0..2 on the tensor engine; tap 3 on DVE
TQ = 4 * TJ  # 2048-col PSUM quad
NWD = NCHUNK * KPE + 1  # 12 diag blocks for PE taps 0-2, + chunk3 tap 3
NWARM = 4  # dummy matmuls to pre-ramp the PE clock
ECHUNK = 2  # this chunk's quads run taps 0-1 on PE, taps 2-3 on DVE

_compiled = None


def _build():
    import concourse.bacc as bacc
    import concourse.mybir as mybir
    from concourse.tile import TileContext

    f32 = mybir.dt.float32
    f16 = mybir.dt.float16
    add = mybir.AluOpType.add
    mult = mybir.AluOpType.mult
    nc = bacc.Bacc(enable_partition_id=False)

    # chunk0's 3 diag blocks + x0's first 515 cols fused into ONE buffer:
    # one DMA -> one completion semaphore gates the first LDWEIGHTS+matmul
    wx0_d = nc.declare_dram_parameter(
        "wx0", [P, KPE * P + TJ + K - 1], f16, isOutput=False
    )
    # remaining diag blocks: chunks 1-3 taps 0-2, then chunk3 tap 3
    wd_d = nc.declare_dram_parameter("wd", [P, (NWD - KPE) * P], f16, isOutput=False)
    # per-partition weight columns: [tap3 for chunks 0-3, tap2 for chunks 0-3]
    wc_d = nc.declare_dram_parameter("wc", [P, 2 * NCHUNK], f32, isOutput=False)
    xw_d = nc.declare_dram_parameter("xw", [P, NCHUNK * TP], f16, isOutput=False)
    out_d = nc.declare_dram_parameter("out", [C, T], f16, isOutput=True)

    with TileContext(nc) as tc:
        with (
            tc.tile_pool(name="xpool", bufs=1) as xpool,
            tc.tile_pool(name="wpool", bufs=1) as wpool,
            tc.tile_pool(name="tpool", bufs=2) as tpool,
            tc.tile_pool(name="spool", bufs=2) as spool,
            tc.tile_pool(name="opool", bufs=2) as opool,
            tc.tile_pool(name="ppool", bufs=2, space="PSUM") as ppool,
        ):
            # PSUM quads: pre-allocate the first so warm-up matmuls can
            # target it (all 8 banks are taken by the 2 quad bufs)
            pts = [ppool.tile([P, TQ], f32, name="pt", tag="pt") for _ in range(2)]

            # --- PE warm-up: no-dep matmuls ramp the clock while DMAs fly
            wux = wpool.tile([P, TJ], f16, tag="wux")
            nc.vector.memset(wux, 0)
            for _ in range(NWARM):
                nc.tensor.matmul(
                    pts[0][:, :TJ], wux[:, :P], wux, start=True, stop=True
                )

            # --- input DMAs, ordered for earliest PE start ---
            h0 = TJ + K - 1  # 515: j-tile 0 + halo
            hm = 2 * TJ + K - 1  # 1027
            h1 = 4 * TJ + K - 1  # 2051: j-tiles 1..3
            wx0 = wpool.tile([P, KPE * P + h0], f16, tag="wx0")
            wd = wpool.tile([P, (NWD - KPE) * P], f16, tag="wd")
            wc = wpool.tile([P, 2 * NCHUNK], f32, tag="wc")

            xts = []
            xt0 = xpool.tile([P, TP], f16, name="xt0", tag="xt0")
            # all input triggers on the sync DGE queue, serially: chunk 0's
            # pieces stay ahead of the bulk weights in descriptor order.
            # First trigger carries chunk0's stationaries + j-tile 0 in one
            # transfer (one completion semaphore for the first matmul).
            nc.sync.dma_start(out=wx0, in_=wx0_d[:, :])
            nc.sync.dma_start(out=xt0[:, TJ:hm], in_=xw_d[:, TJ:hm])
            nc.sync.dma_start(out=xt0[:, hm:h1], in_=xw_d[:, hm:h1])
            nc.sync.dma_start(out=xt0[:, h1:], in_=xw_d[:, h1:TP])
            nc.sync.dma_start(out=wc, in_=wc_d[:, :])
            nc.sync.dma_start(out=wd, in_=wd_d[:, :])
            xts.append(xt0)
            # chunks 1-3 in 2 pieces each
            for c in range(1, NCHUNK):
                xt = xpool.tile([P, TP], f16, name=f"xt{c}", tag=f"xt{c}")
                base = c * TP
                nc.sync.dma_start(out=xt[:, :h1], in_=xw_d[:, base : base + h1])
                nc.sync.dma_start(out=xt[:, h1:], in_=xw_d[:, base + h1 : base + TP])
                xts.append(xt)

            qidx = 0
            for chunk in range(NCHUNK):
                xv = xts[chunk]
                ot = opool.tile([P, T], f16, tag="ot")
                w3c = wc[:, chunk : chunk + 1]
                w2c = wc[:, NCHUNK + chunk : NCHUNK + chunk + 1]
                last_chunk = chunk == NCHUNK - 1
                # tap 3 on DVE, in pieces matching the x-load pieces;
                # chunk 3 needs none (q0 runs 4 taps on PE, q1 folds tap 3
                # into its stt evacuation)
                tt = None
                if not last_chunk:
                    tt = tpool.tile([P, T], f16, tag="tt")
                    if chunk == 0:
                        # j-tile 0's tap-3 slice lives in the fused wx0 tile
                        nc.vector.tensor_scalar_mul(
                            tt[:, 0:TJ],
                            wx0[:, KPE * P + K - 1 : KPE * P + K - 1 + TJ],
                            w3c,
                        )
                        pieces = [(TJ, TQ), (TQ, T)]
                    else:
                        pieces = [(0, TQ), (TQ, T)]
                    for lo, hi in pieces:
                        nc.vector.tensor_scalar_mul(
                            tt[:, lo:hi], xv[:, lo + K - 1 : hi + K - 1], w3c
                        )
                for q in range(2):
                    qlo = q * TQ
                    tail = last_chunk and q == 1
                    pe4 = last_chunk and q == 0
                    # only chunk 2 shifts tap 2 to the DVE: more 2-tap quads
                    # make the PE outrun the ACT evacuation pipeline (2 PSUM
                    # bufs) and stall on PSUM banks
                    echunk = chunk == ECHUNK
                    ntap = 4 if pe4 else (2 if echunk else KPE)
                    if qidx < 2:
                        pt = pts[qidx]  # pre-allocated (warm-up target)
                    else:
                        pt = ppool.tile([P, TQ], f32, name="pt", tag="pt")
                    qidx += 1
                    for s in range(4):
                        ntap_s = ntap
                        for k in range(ntap_s):
                            col = qlo + s * TJ + k
                            if chunk == 0:
                                stat = wx0[:, k * P : (k + 1) * P]
                            else:
                                # chunk3 tap3 diag lives in the last block
                                blk = (
                                    NWD - KPE - 1
                                    if k == KPE
                                    else (chunk - 1) * KPE + k
                                )
                                stat = wd[:, blk * P : (blk + 1) * P]
                            if chunk == 0 and col < TJ:
                                mov = wx0[:, KPE * P + k : KPE * P + k + TJ]
                            else:
                                mov = xv[:, col : col + TJ]
                            nc.tensor.matmul(
                                pt[:, s * TJ : (s + 1) * TJ],
                                stat,
                                mov,
                                start=(k == 0),
                                stop=(k == ntap_s - 1),
                            )
                    if tail:
                        # out = x3*w3 + psum in 512-col pieces on DVE, into a
                        # dedicated tile; each piece's sync-DGE trigger and
                        # transfer pipeline behind the next stt
                        ott = tpool.tile([P, TQ], f16, tag="ott")
                        for s in range(4):
                            lo = qlo + s * TJ
                            nc.vector.scalar_tensor_tensor(
                                out=ott[:, s * TJ : (s + 1) * TJ],
                                in0=xv[:, lo + K - 1 : lo + K - 1 + TJ],
                                scalar=w3c,
                                in1=pt[:, s * TJ : (s + 1) * TJ],
                                op0=mult,
                                op1=add,
                            )
                            nc.sync.dma_start(
                                out=out_d[chunk * P : (chunk + 1) * P, lo : lo + TJ],
                                in_=ott[:, s * TJ : (s + 1) * TJ],
                            )
                    elif pe4:
                        # 4-tap PSUM: ACT-only evacuation, out via ACT DGE
                        nc.scalar.copy(ot[:, qlo : qlo + TQ], pt)
                        nc.scalar.dma_start(
                            out=out_d[chunk * P : (chunk + 1) * P, qlo : qlo + TQ],
                            in_=ot[:, qlo : qlo + TQ],
                        )
                    else:
                        if echunk:
                            # taps 2+3 on DVE: t23 = x2*w2 + tt
                            t23 = spool.tile([P, TQ], f16, tag="t23")
                            nc.vector.tensor_scalar_mul(
                                t23, xv[:, qlo + 2 : qlo + 2 + TQ], w2c
                            )
                            nc.vector.tensor_tensor(
                                out=t23, in0=t23, in1=tt[:, qlo : qlo + TQ], op=add
                            )
                            tsum = t23
                        else:
                            tsum = tt[:, qlo : qlo + TQ]
                        st = spool.tile([P, TQ], f16, tag="st")
                        nc.scalar.copy(st, pt)
                        nc.vector.tensor_tensor(
                            out=ot[:, qlo : qlo + TQ], in0=st, in1=tsum, op=add
                        )
                        nc.sync.dma_start(
                            out=out_d[chunk * P : (chunk + 1) * P, qlo : qlo + TQ],
                            in_=ot[:, qlo : qlo + TQ],
                        )

    nc.compile()
    return nc


def _prep_inputs(x: np.ndarray, weight: np.ndarray):
    # diag stationary tiles for taps 0..2: block (c*KPE + k) is
    # diag(weight[c*P:(c+1)*P, k]) as [P, P] f16; one extra block is
    # chunk 3's tap-3 diag (used by the 4-tap second-to-last quad)
    wdall = np.zeros((P, NWD * P), dtype=np.float16)
    idx = np.arange(P)
    for c in range(NCHUNK):
        for k in range(KPE):
            blk = (c * KPE + k) * P
            wdall[idx, blk + idx] = weight[c * P : (c + 1) * P, k].astype(np.float16)
    wdall[idx, (NWD - 1) * P + idx] = weight[(NCHUNK - 1) * P :, 3].astype(np.float16)
    wd = np.ascontiguousarray(wdall[:, KPE * P :])  # chunks 1-3 + c3 tap3
    # per-partition tap-3 and tap-2 columns, kept f32 (the scalar operand
    # is exempt from the DVE 2x dtype rule)
    wc = np.concatenate(
        [
            weight[:, 3].reshape(NCHUNK, P).T.astype(np.float32),
            weight[:, 2].reshape(NCHUNK, P).T.astype(np.float32),
        ],
        axis=1,
    )
    wc = np.ascontiguousarray(wc)
    h0 = TJ + K - 1
    maps = []
    for b in range(B):
        xp = np.zeros((C, TP), dtype=np.float32)
        xp[:, K - 1 :] = x[b].T  # [512, 4099], 3 leading zeros
        xw = np.ascontiguousarray(
            xp.reshape(NCHUNK, P, TP).transpose(1, 0, 2).reshape(P, NCHUNK * TP)
        ).astype(np.float16)
        # fused first transfer: chunk0's 3 diag blocks + x0's first 515 cols
        wx0 = np.ascontiguousarray(
            np.concatenate([wdall[:, : KPE * P], xw[:, :h0]], axis=1)
        )
        maps.append({"xw": xw, "wx0": wx0, "wd": wd, "wc": wc})
    return maps


def _ensure_axon_hooks():
    """This image's antenv package lacks axon_hooks; synthesize it so a
    trace=True / BASS_TRACE run of run_bass_kernel_spmd can profile
    instead of crashing on import."""
    import sys
    import types

    if "antenv.axon_hooks" in sys.modules:
        return
    mod = types.ModuleType("antenv.axon_hooks")
    state = {"hook": None}
    mod.set_axon_ntff_profile_hook = lambda h: state.__setitem__("hook", h)
    mod.get_axon_ntff_profile_hook = lambda: state["hook"]
    sys.modules["antenv.axon_hooks"] = mod
    try:
        if "/root/.axon_site" not in sys.path:
            sys.path.insert(0, "/root/.axon_site")
        from trn_agent_boot.trn_boot import _ntff_profile_via_ctypes

        mod.set_axon_ntff_profile_hook(
            _ntff_profile_via_ctypes("/opt/axon/libaxon_pjrt.so")
        )
    except Exception:
        pass  # hook stays None; concourse degrades to no-trace


def kernel(x: np.ndarray, weight: np.ndarray) -> np.ndarray:
    global _compiled
    _ensure_axon_hooks()
    from concourse import bass_utils

    x = np.ascontiguousarray(x, dtype=np.float32)
    weight = np.ascontiguousarray(weight, dtype=np.float32)

    if _compiled is None:
        _compiled = _build()
    nc = _compiled

    in_maps = _prep_inputs(x, weight)
    res = bass_utils.run_bass_kernel_spmd(nc, in_maps, core_ids=list(range(B)))

    out = np.empty((B, T, C), dtype=np.float32)
    for b in range(B):
        out[b] = np.asarray(res.results[b]["out"]).astype(np.float32).T
    return out


# revision 38
# speedup vs baseline: 1.0233x; 1.0233x over previous
"""Depthwise causal conv1d kernel for Trainium2 (8 NeuronCores, SPMD).

Problem: x [B=8, T=4096, C=512] f32, weight [C=512, K=4] f32.
out[b, t, c] = sum_k weight[c, k] * x[b, t - 3 + k, c]   (causal, zero-pad)

Strategy (v6 — engine-split, warmed PE, 2048-col PSUM quads):
  - Data-parallel over batch: core b handles x[b].
  - Host-side layout: channels-first x[b].T padded with 3 leading zeros
    along time -> [C=512, T+3=4099] fp16, chunk-blocked to [128, 4*4099].
    fp16 halves HBM traffic; accumulation is fp32 in PSUM / DVE ALU.
  - Diag stationary matrices are built on the HOST and DMA'd.
  - Work split: PE does taps 0-2 as accumulating diag matmuls (chunk 2's
    quads run only taps 0-1 on the PE, with taps 2-3 on DVE, trimming
    the PE's critical path), DVE does tap 3 via tensor_scalar_mul (4x
    fp16 mode) plus the final f16 adds (2x mode), ACT evacuates
    PSUM->f16 in 2048-col instructions. PSUM = 2 x [128, 2048] quads =
    all 8 banks.
  - A short burst of dummy matmuls (fed by the first-emitted DVE memset,
    writing the pre-allocated first PSUM quad) ramps the PE p-state to
    full speed while the input DMAs are in flight, so every real matmul
    runs at ~216 ns instead of paying the 3 us mid-speed ramp.
  - Tail: chunk3-q0 runs all 4 taps on PE with an ACT-only evacuation
    (copy + ACT-DGE trigger); the last quad evacuates via 4x 512-col
    scalar_tensor_tensor (out = x3*w3 + psum) into a DEDICATED output
    tile (avoids a tile-ordering false dependency on chunk3's other
    output) with sync-DGE triggers, which by then are idle.
"""

import numpy as np

B, T, C, K = 8, 4096, 512, 4
P = 128  # partitions
NCHUNK = C // P  # 4 channel chunks
TJ = 512  # time-tile (free dim) per matmul; max moving free dim
NJ = T // TJ  # 8 time tiles per chunk
TP = T + K - 1  # padded time = 4099
KPE = 3  # taps 0..2 on the tensor engine; tap 3 on DVE
TQ = 4 * TJ  # 2048-col PSUM quad
NWD = NCHUNK * KPE + 1  # 12 diag blocks for PE taps 0-2, + chunk3 tap 3
NWARM = 4  # dummy matmuls to pre-ramp the PE clock
ECHUNK = 2  # this chunk's quads run taps 0-1 on PE, taps 2-3 on DVE

_compiled = None


def _build():
    import concourse.bacc as bacc
    import concourse.mybir as mybir
    from concourse.tile import TileContext

    f32 = mybir.dt.float32
    f16 = mybir.dt.float16
    add = mybir.AluOpType.add
    mult = mybir.AluOpType.mult
    nc = bacc.Bacc(enable_partition_id=False)

    wd_d = nc.declare_dram_parameter("wd", [P, NWD * P], f16, isOutput=False)
    # per-partition weight columns: [tap3 for chunks 0-3, tap2 for chunks 0-3]
    wc_d = nc.declare_dram_parameter("wc", [P, 2 * NCHUNK], f32, isOutput=False)
    xw_d = nc.declare_dram_parameter("xw", [P, NCHUNK * TP], f16, isOutput=False)
    out_d = nc.declare_dram_parameter("out", [C, T], f16, isOutput=True)

    with TileContext(nc) as tc:
        with (
            tc.tile_pool(name="xpool", bufs=1) as xpool,
            tc.tile_pool(name="wpool", bufs=1) as wpool,
            tc.tile_pool(name="tpool", bufs=2) as tpool,
            tc.tile_pool(name="spool", bufs=2) as spool,
            tc.tile_pool(name="opool", bufs=2) as opool,
            tc.tile_pool(name="ppool", bufs=2, space="PSUM") as ppool,
        ):
            # PSUM quads: pre-allocate the first so warm-up matmuls can
            # target it (all 8 banks are taken by the 2 quad bufs)
            pts = [ppool.tile([P, TQ], f32, name="pt", tag="pt") for _ in range(2)]

            # --- PE warm-up: no-dep matmuls ramp the clock while DMAs fly
            wux = wpool.tile([P, TJ], f16, tag="wux")
            nc.vector.memset(wux, 0)
            for _ in range(NWARM):
                nc.tensor.matmul(
                    pts[0][:, :TJ], wux[:, :P], wux, start=True, stop=True
                )

            # --- input DMAs, ordered for earliest PE start ---
            wd = wpool.tile([P, NWD * P], f16, tag="wd")
            wc = wpool.tile([P, 2 * NCHUNK], f32, tag="wc")

            xts = []
            # chunk 0 in 4 fine pieces so the PE never stalls early
            xt0 = xpool.tile([P, TP], f16, name="xt0", tag="xt0")
            h0 = TJ + K - 1  # 515: j-tile 0 + halo
            hm = 2 * TJ + K - 1  # 1027
            h1 = 4 * TJ + K - 1  # 2051: j-tiles 1..3
            # all input triggers on the sync DGE queue, serially: chunk 0's
            # pieces stay ahead of the bulk weights in descriptor order
            # (firing them concurrently from two queues congests the early
            # stream and lands chunk 0 late)
            nc.sync.dma_start(out=xt0[:, :h0], in_=xw_d[:, 0:h0])
            nc.sync.dma_start(out=wd[:, : KPE * P], in_=wd_d[:, : KPE * P])
            nc.sync.dma_start(out=xt0[:, h0:hm], in_=xw_d[:, h0:hm])
            nc.sync.dma_start(out=xt0[:, hm:h1], in_=xw_d[:, hm:h1])
            nc.sync.dma_start(out=xt0[:, h1:], in_=xw_d[:, h1:TP])
            nc.sync.dma_start(out=wc, in_=wc_d[:, :])
            nc.sync.dma_start(out=wd[:, KPE * P :], in_=wd_d[:, KPE * P :])
            xts.append(xt0)
            # chunks 1-3 in 2 pieces each
            for c in range(1, NCHUNK):
                xt = xpool.tile([P, TP], f16, name=f"xt{c}", tag=f"xt{c}")
                base = c * TP
                nc.sync.dma_start(out=xt[:, :h1], in_=xw_d[:, base : base + h1])
                nc.sync.dma_start(out=xt[:, h1:], in_=xw_d[:, base + h1 : base + TP])
                xts.append(xt)

            qidx = 0
            for chunk in range(NCHUNK):
                xv = xts[chunk]
                ot = opool.tile([P, T], f16, tag="ot")
                w3c = wc[:, chunk : chunk + 1]
                w2c = wc[:, NCHUNK + chunk : NCHUNK + chunk + 1]
                last_chunk = chunk == NCHUNK - 1
                # tap 3 on DVE, in pieces matching the x-load pieces;
                # chunk 3 needs none (q0 runs 4 taps on PE, q1 folds tap 3
                # into its stt evacuation)
                tt = None
                if not last_chunk:
                    tt = tpool.tile([P, T], f16, tag="tt")
                    if chunk == 0:
                        pieces = [(0, TJ), (TJ, TQ), (TQ, T)]
                    else:
                        pieces = [(0, TQ), (TQ, T)]
                    for lo, hi in pieces:
                        nc.vector.tensor_scalar_mul(
                            tt[:, lo:hi], xv[:, lo + K - 1 : hi + K - 1], w3c
                        )
                for q in range(2):
                    qlo = q * TQ
                    tail = last_chunk and q == 1
                    pe4 = last_chunk and q == 0
                    # only chunk 2 shifts tap 2 to the DVE: more 2-tap quads
                    # make the PE outrun the ACT evacuation pipeline (2 PSUM
                    # bufs) and stall on PSUM banks
                    echunk = chunk == ECHUNK
                    ntap = 4 if pe4 else (2 if echunk else KPE)
                    if qidx < 2:
                        pt = pts[qidx]  # pre-allocated (warm-up target)
                    else:
                        pt = ppool.tile([P, TQ], f32, name="pt", tag="pt")
                    qidx += 1
                    for s in range(4):
                        ntap_s = ntap
                        for k in range(ntap_s):
                            # chunk3 tap3 diag lives in the extra block 12
                            blk = NWD - 1 if k == KPE else chunk * KPE + k
                            col = qlo + s * TJ + k
                            nc.tensor.matmul(
                                pt[:, s * TJ : (s + 1) * TJ],
                                wd[:, blk * P : (blk + 1) * P],
                                xv[:, col : col + TJ],
                                start=(k == 0),
                                stop=(k == ntap_s - 1),
                            )
                    if tail:
                        # out = x3*w3 + psum in 512-col pieces on DVE, into a
                        # dedicated tile; each piece's sync-DGE trigger and
                        # transfer pipeline behind the next stt
                        ott = tpool.tile([P, TQ], f16, tag="ott")
                        for s in range(4):
                            lo = qlo + s * TJ
                            nc.vector.scalar_tensor_tensor(
                                out=ott[:, s * TJ : (s + 1) * TJ],
                                in0=xv[:, lo + K - 1 : lo + K - 1 + TJ],
                                scalar=w3c,
                                in1=pt[:, s * TJ : (s + 1) * TJ],
                                op0=mult,
                                op1=add,
                            )
                            nc.sync.dma_start(
                                out=out_d[chunk * P : (chunk + 1) * P, lo : lo + TJ],
                                in_=ott[:, s * TJ : (s + 1) * TJ],
                            )
                    elif pe4:
                        # 4-tap PSUM: ACT-only evacuation, out via ACT DGE
                        nc.scalar.copy(ot[:, qlo : qlo + TQ], pt)
                        nc.scalar.dma_start(
                            out=out_d[chunk * P : (chunk + 1) * P, qlo : qlo + TQ],
                            in_=ot[:, qlo : qlo + TQ],
                        )
                    else:
                        if echunk:
                            # taps 2+3 on DVE: t23 = x2*w2 + tt
                            t23 = spool.tile([P, TQ], f16, tag="t23")
                            nc.vector.tensor_scalar_mul(
                                t23, xv[:, qlo + 2 : qlo + 2 + TQ], w2c
                            )
                            nc.vector.tensor_tensor(
                                out=t23, in0=t23, in1=tt[:, qlo : qlo + TQ], op=add
                            )
                            tsum = t23
                        else:
                            tsum = tt[:, qlo : qlo + TQ]
                        st = spool.tile([P, TQ], f16, tag="st")
                        nc.scalar.copy(st, pt)
                        nc.vector.tensor_tensor(
                            out=ot[:, qlo : qlo + TQ], in0=st, in1=tsum, op=add
                        )
                        nc.sync.dma_start(
                            out=out_d[chunk * P : (chunk + 1) * P, qlo : qlo + TQ],
                            in_=ot[:, qlo : qlo + TQ],
                        )

    nc.compile()
    return nc


def _prep_inputs(x: np.ndarray, weight: np.ndarray):
    # diag stationary tiles for taps 0..2: block (c*KPE + k) is
    # diag(weight[c*P:(c+1)*P, k]) as [P, P] f16; block 12 is chunk 3's
    # tap-3 diag (used by the 4-tap second-to-last quad)
    wd = np.zeros((P, NWD * P), dtype=np.float16)
    idx = np.arange(P)
    for c in range(NCHUNK):
        for k in range(KPE):
            blk = (c * KPE + k) * P
            wd[idx, blk + idx] = weight[c * P : (c + 1) * P, k].astype(np.float16)
    wd[idx, (NWD - 1) * P + idx] = weight[(NCHUNK - 1) * P :, 3].astype(np.float16)
    # per-partition tap-3 and tap-2 columns, kept f32 (the scalar operand
    # is exempt from the DVE 2x dtype rule)
    wc = np.concatenate(
        [
            weight[:, 3].reshape(NCHUNK, P).T.astype(np.float32),
            weight[:, 2].reshape(NCHUNK, P).T.astype(np.float32),
        ],
        axis=1,
    )
    wc = np.ascontiguousarray(wc)
    xs = []
    for b in range(B):
        xp = np.zeros((C, TP), dtype=np.float32)
        xp[:, K - 1 :] = x[b].T  # [512, 4099], 3 leading zeros
        xw = np.ascontiguousarray(
            xp.reshape(NCHUNK, P, TP).transpose(1, 0, 2).reshape(P, NCHUNK * TP)
        ).astype(np.float16)
        xs.append(xw)
    return xs, wd, wc


def _ensure_axon_hooks():
    """This image's antenv package lacks axon_hooks; synthesize it so a
    trace=True / BASS_TRACE run of run_bass_kernel_spmd can profile
    instead of crashing on import."""
    import sys
    import types

    if "antenv.axon_hooks" in sys.modules:
        return
    mod = types.ModuleType("antenv.axon_hooks")
    state = {"hook": None}
    mod.set_axon_ntff_profile_hook = lambda h: state.__setitem__("hook", h)
    mod.get_axon_ntff_profile_hook = lambda: state["hook"]
    sys.modules["antenv.axon_hooks"] = mod
    try:
        if "/root/.axon_site" not in sys.path:
            sys.path.insert(0, "/root/.axon_site")
        from trn_agent_boot.trn_boot import _ntff_profile_via_ctypes

        mod.set_axon_ntff_profile_hook(
            _ntff_profile_via_ctypes("/opt/axon/libaxon_pjrt.so")
        )
    except Exception:
        pass  # hook stays None; concourse degrades to no-trace


def kernel(x: np.ndarray, weight: np.ndarray) -> np.ndarray:
    global _compiled
    _ensure_axon_hooks()
    from concourse import bass_utils

    x = np.ascontiguousarray(x, dtype=np.float32)
    weight = np.ascontiguousarray(weight, dtype=np.float32)

    if _compiled is None:
        _compiled = _build()
    nc = _compiled

    xs, wd, wc = _prep_inputs(x, weight)
    in_maps = [{"xw": xs[b], "wd": wd, "wc": wc} for b in range(B)]
    res = bass_utils.run_bass_kernel_spmd(nc, in_maps, core_ids=list(range(B)))

    out = np.empty((B, T, C), dtype=np.float32)
    for b in range(B):
        out[b] = np.asarray(res.results[b]["out"]).astype(np.float32).T
    return out
